# revision 1
# baseline (speedup 1.0000x reference)
"""Trainium2 Bass kernel for nn_BaseLineModel (segment_reduce).

Model: token-embed -> conv1d(K=3) -> relu -> global max-pool per note,
concat with delta-times, segment-mean over notes sharing a start day,
tiny linear + sigmoid -> [S, 1].

Sharding: notes (N=16384) split 8 ways. Per core: gather embeddings for
2048 notes (dma_gather transpose mode, bf16, rows padded to 256B), conv
as 2 PSUM-accumulated matmuls (contraction (e,k0|k1)=128 via an SBUF
shifted-copy stack, plus (e,k2)=64 as an AP view), DVE max-pool, ACT
relu+bias, PE transpose to note-major, segment-sum over S=1024 days via
one-hot float32r matmuls, ReduceScatter(add) across the 8 cores, and the
final mean/linear/sigmoid on each core's 128-day slice.
"""

import numpy as np
import ml_dtypes

import concourse.bass as bass
import concourse.mybir as mybir
import concourse.tile as tile
from concourse.bass_utils import run_bass_kernel_spmd
from concourse import library_config
from concourse.masks import make_identity
from concourse.vector_clock import ScopedClock  # noqa: F401  (import sanity)

# ---- problem dims (hardcoded per task contract) ----
N, L, E, H, K, V, S = 16384, 64, 64, 256, 3, 30000, 1024
NCORES = 8
NC_NOTES = N // NCORES            # 2048 notes per core
NTOK = NC_NOTES * L               # 131072 tokens per core
BLK_NOTES = 128                   # notes per gather block
BLK_TOK = BLK_NOTES * L           # 8192 tokens per block
NBLK = NC_NOTES // BLK_NOTES      # 16
GRP_NOTES = 8                     # notes per matmul group (512 cols)
NGRP = BLK_NOTES // GRP_NOTES     # 16 groups per block
GCHUNK = 512                      # tokens per dma_gather call (desc-ring cap)
NGC = BLK_TOK // GCHUNK           # gather calls per block
NCHUNK = NC_NOTES // 128          # 16 note-chunks for segment phase
F = 258                           # main cols: delta | 256 feats | ones
TMAX = L - K + 1                  # 62 valid conv positions

_SPLIT_MAXW = 1


def _split_waits(nc, maxw=_SPLIT_MAXW):
    """This walrus build rejects >1 sync wait per instruction; move extras
    onto preceding same-engine NOPs (sequencer order preserves semantics)."""
    for bb in nc.main_func.blocks:
        out = []
        for inst in bb.instructions:
            si = inst.sync_info
            waits = list(si.on_wait) if (si is not None and si.on_wait) else []
            if len(waits) > maxw:
                rest = waits[:-maxw]
                si.on_wait = waits[-maxw:]
                for i in range(0, len(rest), maxw):
                    out.append(mybir.InstNoOp(
                        name=f"{inst.name}-wsplit{i}",
                        sync_info=mybir.SyncInfo(on_wait=rest[i:i + maxw], on_update=[]),
                        bass_nofuse=True,
                        engine=inst.engine,
                    ))
            out.append(inst)
        bb.instructions = out


def _build_nc(reps=1, use_cc=True, mode='full'):
    f32 = mybir.dt.float32
    f32r = mybir.dt.float32r
    bf16 = mybir.dt.bfloat16
    i16 = mybir.dt.int16

    nc = bass.Bass()
    d_embp = nc.declare_dram_parameter("embp", [V, 128], bf16, isOutput=False)
    d_idx = nc.declare_dram_parameter("idx", [NBLK, 128, BLK_TOK // 16], i16, isOutput=False)
    d_stf = nc.declare_dram_parameter("stf", [NC_NOTES, 1], f32, isOutput=False)
    d_delta = nc.declare_dram_parameter("delta", [NC_NOTES, 1], f32, isOutput=False)
    d_w01 = nc.declare_dram_parameter("w01", [128, H], bf16, isOutput=False)
    d_w2 = nc.declare_dram_parameter("w2", [64, H], bf16, isOutput=False)
    d_cb = nc.declare_dram_parameter("convb2", [128, 2], f32, isOutput=False)
    d_iota = nc.declare_dram_parameter("iota", [128, S], f32, isOutput=False)
    d_wrep = nc.declare_dram_parameter("wrep", [128, H + 1], f32, isOutput=False)
    d_brep = nc.declare_dram_parameter("brep", [128, 1], f32, isOutput=False)
    d_out = nc.declare_dram_parameter("out", [128, 1], f32, isOutput=True)
    part = nc.dram_tensor("part", [S, F], f32)
    rs_out = nc.dram_tensor("rs_out", [128, F], f32)

    with tile.TileContext(nc) as tc:
        nc.gpsimd.load_library(library_config.mlp)
        nidx_reg_cm = nc.gpsimd.register("nidx")
        nidx_reg = nidx_reg_cm.__enter__()
        nc.gpsimd.reg_mov(nidx_reg, GCHUNK)
        rep_range = range(reps)
        with (
            tc.tile_pool(name="cst", bufs=1) as cp,
            tc.tile_pool(name="feat", bufs=1) as fp,
            tc.tile_pool(name="mainp", bufs=1) as mp,
        ):
         for _rep in rep_range:
             w01_sb = cp.tile([128, H], bf16)
             w2_sb = cp.tile([64, H], bf16)
             cb_sb = cp.tile([128, 2], f32)
             ident = cp.tile([128, 128], f32)
             nc.sync.dma_start(out=w01_sb[:], in_=d_w01[:])
             nc.sync.dma_start(out=w2_sb[:], in_=d_w2[:])
             nc.sync.dma_start(out=cb_sb[:], in_=d_cb[:])
             make_identity(nc, ident[:])
             feats = [fp.tile([128, NC_NOTES], f32, name=f"feats{hh}") for hh in range(2)]

             # ---- P1: gather + conv + maxpool, per block ----
             with (
                 tc.tile_pool(name="gath", bufs=3) as gp,
                 tc.tile_pool(name="ypsum", bufs=6, space="PSUM") as yp,
             ):
                 for b in range(NBLK):
                     idx_sb = gp.tile([128, BLK_TOK // 16], i16, tag="idx")
                     nc.sync.dma_start(out=idx_sb[:], in_=d_idx[b])
                     x_sb = gp.tile([128, BLK_TOK], bf16, tag="x")
                     for c in range(NGC):
                         nc.gpsimd.dma_gather(
                             out_ap=x_sb[:, c * GCHUNK:(c + 1) * GCHUNK]
                                 .rearrange("p (o n) -> p o n", o=1),
                             in_ap=d_embp[:],
                             idxs_ap=idx_sb[:, c * (GCHUNK // 16):(c + 1) * (GCHUNK // 16)],
                             num_idxs=GCHUNK,
                             num_idxs_reg=nidx_reg,
                             elem_size=128,
                             transpose=True,
                         )
                     # stack k=1 shift into partitions 64:128 (SBUF->SBUF DMA)
                     nc.sync.dma_start(out=x_sb[64:128, 0:BLK_TOK - 1],
                                       in_=x_sb[0:64, 1:BLK_TOK])
                     for g in (range(NGRP) if mode != 'gather' else ()):
                         c0 = g * 512
                         for hh in range(2):
                             y_ps = yp.tile([128, 512], f32, tag="y", name=f"y{b}_{g}_{hh}")
                             nc.tensor.matmul(out=y_ps[:],
                                              lhsT=w01_sb[:, hh * 128:(hh + 1) * 128],
                                              rhs=x_sb[:, c0:c0 + 512],
                                              start=True, stop=False)
                             nc.tensor.matmul(out=y_ps[:, 0:510],
                                              lhsT=w2_sb[:, hh * 128:(hh + 1) * 128],
                                              rhs=x_sb[0:64, c0 + 2:c0 + 512],
                                              start=False, stop=True)
                             nc.vector.reduce_max(
                                 out=feats[hh][:, b * BLK_NOTES + g * 8:
                                               b * BLK_NOTES + g * 8 + 8],
                                 in_=y_ps[:].rearrange("p (n l) -> p n l", l=L)[:, :, 0:TMAX],
                                 axis=mybir.AxisListType.X)

             # ---- P2: relu(feats + conv_b) ----
             if mode == 'gather':
                 nc.vector.memset(feats[0][:], 0.0)
                 nc.vector.memset(feats[1][:], 0.0)
             for hh in range(2):
                 nc.scalar.activation(out=feats[hh][:], in_=feats[hh][:],
                                      func=mybir.ActivationFunctionType.Relu,
                                      bias=cb_sb[:, hh:hh + 1], scale=1.0)

             # ---- P3: transpose to note-major main tiles ----
             mains = [mp.tile([128, F], f32, name=f"main{i}") for i in range(NCHUNK)]
             with tc.tile_pool(name="tpsum", bufs=2, space="PSUM") as tp:
                 for i in range(NCHUNK):
                     nc.sync.dma_start(out=mains[i][:, 0:1],
                                       in_=d_delta[i * 128:(i + 1) * 128, :])
                     nc.vector.memset(mains[i][:, H + 1:H + 2], 1.0)
                     for hh in range(2):
                         t_ps = tp.tile([128, 128], f32, tag="t", name=f"t{i}_{hh}")
                         nc.tensor.transpose(out=t_ps[:],
                                             in_=feats[hh][:, i * 128:(i + 1) * 128],
                                             identity=ident[:])
                         nc.vector.tensor_copy(
                             out=mains[i][:, 1 + hh * 128:1 + (hh + 1) * 128],
                             in_=t_ps[:])

             # ---- P4: segment-sum via one-hot matmuls ----
             with (
                 tc.tile_pool(name="segsb", bufs=2) as ssp,
                 tc.tile_pool(name="segps", bufs=1, space="PSUM") as pp,
             ):
                 iota_sb = cp.tile([128, S], f32)
                 nc.sync.dma_start(out=iota_sb[:], in_=d_iota[:])
                 seg_ps = [pp.tile([128, F], f32, tag=f"seg{bk}", name=f"seg{bk}")
                           for bk in range(8)]
                 for i in range(NCHUNK):
                     st_sb = ssp.tile([128, 1], f32, tag="st")
                     nc.sync.dma_start(out=st_sb[:], in_=d_stf[i * 128:(i + 1) * 128, :])
                     oh_sb = ssp.tile([128, S], f32, tag="oh")
                     nc.vector.tensor_tensor(out=oh_sb[:],
                                             in0=st_sb[:, 0:1].to_broadcast([128, S]),
                                             in1=iota_sb[:],
                                             op=mybir.AluOpType.is_equal)
                     for bk in range(8):
                         nc.tensor.matmul(out=seg_ps[bk][:],
                                          lhsT=oh_sb[:, bk * 128:(bk + 1) * 128],
                                          rhs=mains[i][:],
                                          start=(i == 0), stop=(i == NCHUNK - 1))
                 for bk in range(8):
                     seg_sb = ssp.tile([128, F], f32, tag="segout")
                     nc.vector.tensor_copy(out=seg_sb[:], in_=seg_ps[bk][:])
                     nc.sync.dma_start(out=part[bk * 128:(bk + 1) * 128, :], in_=seg_sb[:])

             # ---- P5: cross-core reduce + finalize ----
             if use_cc:
                 with tc.tile_critical():
                     with nc.semaphore("cc_sem") as cc_sem:
                         nc.gpsimd.collective_compute(
                             "ReduceScatter", mybir.AluOpType.add,
                             replica_groups=[list(range(NCORES))],
                             ins=[part[:]], outs=[rs_out[:]],
                         ).then_inc(cc_sem, 1)
                         nc.gpsimd.wait_ge(cc_sem, 1)
             else:
                 nc.sync.dma_start(out=rs_out[:], in_=part[0:128, :])

             with tc.tile_pool(name="fin", bufs=1) as fin:
                 wrep_sb = fin.tile([128, H + 1], f32)
                 nc.sync.dma_start(out=wrep_sb[:], in_=d_wrep[:])
                 brep_sb = fin.tile([128, 1], f32)
                 nc.sync.dma_start(out=brep_sb[:], in_=d_brep[:])
                 fs = fin.tile([128, F], f32)
                 nc.sync.dma_start(out=fs[:], in_=rs_out[:])
                 cnt = fin.tile([128, 1], f32)
                 nc.vector.tensor_scalar_max(out=cnt[:], in0=fs[:, H + 1:H + 2], scalar1=1.0)
                 rcp = fin.tile([128, 1], f32)
                 nc.vector.reciprocal(out=rcp[:], in_=cnt[:])
                 prod = fin.tile([128, H + 1], f32)
                 nc.vector.tensor_tensor(out=prod[:], in0=fs[:, 0:H + 1], in1=wrep_sb[:],
                                         op=mybir.AluOpType.mult)
                 dot = fin.tile([128, 1], f32)
                 nc.vector.reduce_sum(out=dot[:], in_=prod[:], axis=mybir.AxisListType.X)
                 nc.vector.tensor_scalar_mul(out=dot[:], in0=dot[:], scalar1=rcp[:, 0:1])
                 nc.vector.tensor_add(out=dot[:], in0=dot[:], in1=brep_sb[:])
                 outsb = fin.tile([128, 1], f32)
                 nc.scalar.activation(out=outsb[:], in_=dot[:],
                                      func=mybir.ActivationFunctionType.Sigmoid, scale=1.0)
                 nc.sync.dma_start(out=d_out[:], in_=outsb[:])

    _split_waits(nc)
    mybir.codegen_inst_isa_subclasses(nc)
    return nc


_NC_CACHE = {}


def _get_nc(reps=1, use_cc=True, mode='full'):
    key = (reps, use_cc, mode)
    if key not in _NC_CACHE:
        _NC_CACHE[key] = _build_nc(reps, use_cc, mode)
    return _NC_CACHE[key]


def _prep_inputs(text, start_times, emb, conv_w, conv_b, W, b):
    bf16 = ml_dtypes.bfloat16
    text = np.asarray(text)[0]              # [N, L]
    st = np.asarray(start_times)[0].astype(np.int64)   # [N]
    emb = np.asarray(emb, dtype=np.float32)
    conv_w = np.asarray(conv_w, dtype=np.float32)
    conv_b = np.asarray(conv_b, dtype=np.float32)
    W = np.asarray(W, dtype=np.float32)
    b = np.asarray(b, dtype=np.float32)

    embp = np.zeros((V, 128), dtype=bf16)
    embp[:, :E] = emb.astype(bf16)

    w01 = np.zeros((128, H), dtype=bf16)
    w01[:64, :] = conv_w[:, :, 0].T.astype(bf16)
    w01[64:, :] = conv_w[:, :, 1].T.astype(bf16)
    w2 = np.ascontiguousarray(conv_w[:, :, 2].T.astype(bf16))
    convb2 = np.ascontiguousarray(conv_b.reshape(2, 128).T.astype(np.float32))

    iota = np.tile(np.arange(S, dtype=np.float32), (128, 1))
    wrep = np.tile(W[:H + 1, 0], (128, 1)).astype(np.float32)
    brep = np.full((128, 1), b[0], np.float32)

    delta_g = np.concatenate([[0.0], np.diff(st).astype(np.float32)]).astype(np.float32)

    tok = text.astype(np.int16)             # V=30000 < 2**15
    in_maps = []
    for c in range(NCORES):
        sl = slice(c * NC_NOTES, (c + 1) * NC_NOTES)
        t = tok[sl].reshape(NBLK, BLK_TOK // GCHUNK, GCHUNK)
        # per-chunk wrap: [32, 16] -> [16, 32], tiled to 128 partitions
        w = t.reshape(NBLK, BLK_TOK // GCHUNK, GCHUNK // 16, 16)
        w = w.transpose(0, 1, 3, 2)                 # [NBLK, NGC, 16, GCHUNK//16]
        w = np.tile(w, (1, 1, 8, 1))                # [NBLK, NGC, 128, GCHUNK//16]
        idx = np.ascontiguousarray(
            w.transpose(0, 2, 1, 3).reshape(NBLK, 128, BLK_TOK // 16))
        in_maps.append({
            "embp": embp,
            "idx": idx,
            "stf": np.ascontiguousarray(st[sl, None].astype(np.float32)),
            "delta": np.ascontiguousarray(delta_g[sl, None]),
            "w01": w01,
            "w2": w2,
            "convb2": convb2,
            "iota": iota,
            "wrep": wrep,
            "brep": brep,
        })
    return in_maps


def kernel(**inputs) -> np.ndarray:
    nc = _get_nc()
    in_maps = _prep_inputs(**inputs)
    res = run_bass_kernel_spmd(nc, in_maps, list(range(NCORES))).results
    out = np.concatenate([res[c]["out"] for c in range(NCORES)], axis=0)
    return out.astype(np.float32)


if __name__ == "__main__":
    import jax
    import reference
    cpu = jax.devices("cpu")[0]
    with jax.default_device(cpu):
        ins = {k: np.asarray(v) for k, v in reference.setup_inputs().items()}
        exp = np.asarray(reference.reference(**reference.setup_inputs()))
    got = kernel(**ins)
    err = np.abs(got - exp).max()
    rel = err / max(np.abs(exp).max(), 1e-9)
    print("max abs err:", err, "rel:", rel)



# revision 9
# speedup vs baseline: 256.0068x; 256.0068x over previous
"""Trainium2 Bass kernel for nn_BaseLineModel (segment_reduce).

Model: token-embed -> conv1d(K=3) -> relu -> global max-pool per note,
concat with delta-times, segment-mean over notes sharing a start day,
tiny linear + sigmoid -> [S, 1].

Design (per core, measured ~1.06 ms/body via hardware-loop slope):
- Linear folded into the segment sum (means@W = segsum(main@W)/cnt), so
  each note contributes one scalar z = delta*W0 + feats@W[1:] plus a
  count; the segment phase reduces [CAP, 2] instead of [CAP, 258].
- DAY-RANGE sharding: start_times are sorted, so core c takes exactly
  the notes whose start day falls in [128c, 128c+128).  Every day's
  notes live on one core, so there is NO cross-core collective (a
  single 8KB ReduceScatter measures ~300us on this stack - more than a
  quarter of the whole kernel).  Cores pad their note count to CAP=2304
  with dummy notes (token 0, st=-1) that never match the core's iota.
- Embedding gather: gpsimd transpose-mode dma_gather (bf16 rows padded
  to 256B), 512 idx/call (hard ucode cap), split across TWO SWDGE
  queues (queue_num=c%2).  The gather is descriptor-bound at ~195ns per
  16-idx descriptor per queue; 2 queues is the stable maximum (3 gives
  nondeterministic results, 4 wedges the device).
- Conv as 2 PSUM-accumulated matmuls per 512-col group (contraction
  (e,k0|k1)=128 via an SBUF shifted-copy stack + (e,k2)=64 view), DVE
  max-pool over the 62 valid positions, ACT relu+bias.
- z via per-chunk matmuls (lhsT=feats chunk, rhs=W column), one-hot
  is_equal segment matmuls into a single [128, 2] PSUM accumulator,
  mean/linear/sigmoid finalized locally per core.
- 4 x-buffers and 8 PSUM y-banks so gather/PE/DVE pipeline across
  blocks; compute hides almost entirely under the gather.
"""

import numpy as np
import ml_dtypes

import concourse.bass as bass
import concourse.mybir as mybir
import concourse.tile as tile
from concourse.bass_utils import run_bass_kernel_spmd
from concourse import library_config

# ---- problem dims (hardcoded per task contract) ----
N, L, E, H, K, V, S = 16384, 64, 64, 256, 3, 30000, 1024
NCORES = 8
CAP = 2304                        # padded notes per core (day-sharded)
BLK_NOTES = 128                   # notes per gather block
BLK_TOK = BLK_NOTES * L           # 8192 tokens per block
NBLK = CAP // BLK_NOTES           # 18
GRP_NOTES = 8                     # notes per matmul group (512 cols)
NGRP = BLK_NOTES // GRP_NOTES     # 16 groups per block
GCHUNK = 512                      # tokens per dma_gather call (desc-ring cap)
NGC = BLK_TOK // GCHUNK           # gather calls per block
NCHUNK = CAP // 128               # 18 note-chunks for segment phase
SDAYS = S // NCORES               # 128 days owned per core
TMAX = L - K + 1                  # 62 valid conv positions

_SPLIT_MAXW = 1


def _split_waits(nc, maxw=_SPLIT_MAXW):
    """This walrus build rejects >1 sync wait per instruction; move extras
    onto preceding same-engine NOPs (sequencer order preserves semantics)."""
    for bb in nc.main_func.blocks:
        out = []
        for inst in bb.instructions:
            si = inst.sync_info
            waits = list(si.on_wait) if (si is not None and si.on_wait) else []
            if len(waits) > maxw:
                rest = waits[:-maxw]
                si.on_wait = waits[-maxw:]
                for i in range(0, len(rest), maxw):
                    out.append(mybir.InstNoOp(
                        name=f"{inst.name}-wsplit{i}",
                        sync_info=mybir.SyncInfo(on_wait=rest[i:i + maxw], on_update=[]),
                        bass_nofuse=True,
                        engine=inst.engine,
                    ))
            out.append(inst)
        bb.instructions = out


def _build_nc(loop_k=1):
    f32 = mybir.dt.float32
    bf16 = mybir.dt.bfloat16
    i16 = mybir.dt.int16

    nc = bass.Bass(num_swdge_queues=2)
    d_embp = nc.declare_dram_parameter("embp", [V, 128], bf16, isOutput=False)
    d_idx = nc.declare_dram_parameter("idx", [NBLK, 128, BLK_TOK // 16], i16, isOutput=False)
    d_stf = nc.declare_dram_parameter("stf", [CAP, 1], f32, isOutput=False)
    d_dpre = nc.declare_dram_parameter("dpre", [128, NCHUNK], f32, isOutput=False)
    d_w01 = nc.declare_dram_parameter("w01", [128, H], bf16, isOutput=False)
    d_w2 = nc.declare_dram_parameter("w2", [64, H], bf16, isOutput=False)
    d_cb = nc.declare_dram_parameter("convb2", [128, 2], f32, isOutput=False)
    d_wf = nc.declare_dram_parameter("wf", [128, 2], f32, isOutput=False)
    d_iota = nc.declare_dram_parameter("iota", [128, SDAYS], f32, isOutput=False)
    d_bsig = nc.declare_dram_parameter("bsig", [128, 1], f32, isOutput=False)
    d_out = nc.declare_dram_parameter("out", [SDAYS, 1], f32, isOutput=True)

    with tile.TileContext(nc) as tc:
        nc.gpsimd.load_library(library_config.mlp)
        nidx_reg_cm = nc.gpsimd.register("nidx")
        nidx_reg = nidx_reg_cm.__enter__()
        nc.gpsimd.reg_mov(nidx_reg, GCHUNK)

        def body():
            with (
                tc.tile_pool(name="cst", bufs=1) as cp,
                tc.tile_pool(name="feat", bufs=1) as fp,
                tc.tile_pool(name="mainp", bufs=1) as mp,
            ):
                w01_sb = cp.tile([128, H], bf16)
                w2_sb = cp.tile([64, H], bf16)
                cb_sb = cp.tile([128, 2], f32)
                wf_sb = cp.tile([128, 2], f32)
                dpre_sb = cp.tile([128, NCHUNK], f32)
                bsig_sb = cp.tile([128, 1], f32)
                iota_sb = cp.tile([128, SDAYS], f32)
                nc.sync.dma_start(out=w01_sb[:], in_=d_w01[:])
                nc.sync.dma_start(out=w2_sb[:], in_=d_w2[:])
                nc.sync.dma_start(out=cb_sb[:], in_=d_cb[:])
                nc.sync.dma_start(out=wf_sb[:], in_=d_wf[:])
                nc.sync.dma_start(out=dpre_sb[:], in_=d_dpre[:])
                nc.sync.dma_start(out=bsig_sb[:], in_=d_bsig[:])
                nc.sync.dma_start(out=iota_sb[:], in_=d_iota[:])
                feats = [fp.tile([128, CAP], f32, name=f"feats{hh}") for hh in range(2)]

                # ---- P1: gather + conv + maxpool, per block ----
                with (
                    tc.tile_pool(name="gath", bufs=4) as gp,
                    tc.tile_pool(name="ypsum", bufs=8, space="PSUM") as yp,
                ):
                    for b in range(NBLK):
                        idx_sb = gp.tile([128, BLK_TOK // 16], i16, tag="idx")
                        nc.sync.dma_start(out=idx_sb[:], in_=d_idx[b])
                        x_sb = gp.tile([128, BLK_TOK], bf16, tag="x")
                        for c in range(NGC):
                            nc.gpsimd.dma_gather(
                                out_ap=x_sb[:, c * GCHUNK:(c + 1) * GCHUNK]
                                    .rearrange("p (o n) -> p o n", o=1),
                                in_ap=d_embp[:],
                                idxs_ap=idx_sb[:, c * (GCHUNK // 16):(c + 1) * (GCHUNK // 16)],
                                num_idxs=GCHUNK,
                                num_idxs_reg=nidx_reg,
                                elem_size=128,
                                transpose=True,
                                queue_num=c % 2,
                            )
                        # stack k=1 shift into partitions 64:128 (SBUF->SBUF DMA)
                        nc.sync.dma_start(out=x_sb[64:128, 0:BLK_TOK - 1],
                                          in_=x_sb[0:64, 1:BLK_TOK])
                        for g in range(NGRP):
                            c0 = g * 512
                            for hh in range(2):
                                y_ps = yp.tile([128, 512], f32, tag="y", name=f"y{b}_{g}_{hh}")
                                nc.tensor.matmul(out=y_ps[:],
                                                 lhsT=w01_sb[:, hh * 128:(hh + 1) * 128],
                                                 rhs=x_sb[:, c0:c0 + 512],
                                                 start=True, stop=False)
                                nc.tensor.matmul(out=y_ps[:, 0:510],
                                                 lhsT=w2_sb[:, hh * 128:(hh + 1) * 128],
                                                 rhs=x_sb[0:64, c0 + 2:c0 + 512],
                                                 start=False, stop=True)
                                nc.vector.reduce_max(
                                    out=feats[hh][:, b * BLK_NOTES + g * 8:
                                                  b * BLK_NOTES + g * 8 + 8],
                                    in_=y_ps[:].rearrange("p (n l) -> p n l", l=L)[:, :, 0:TMAX],
                                    axis=mybir.AxisListType.X)

                # ---- P2: relu(feats + conv_b) ----
                for hh in range(2):
                    nc.scalar.activation(out=feats[hh][:], in_=feats[hh][:],
                                         func=mybir.ActivationFunctionType.Relu,
                                         bias=cb_sb[:, hh:hh + 1], scale=1.0)

                # ---- P3: per-note scalar z = feats @ Wf (+ delta*W0), note-major ----
                mains_sb = mp.tile([128, 2 * NCHUNK], f32, name="mains")
                nc.vector.memset(
                    mains_sb[:].rearrange("p (i two) -> p i two", two=2)[:, :, 1:2], 1.0)
                with tc.tile_pool(name="zps", bufs=2, space="PSUM") as zp:
                    for i in range(NCHUNK):
                        z_ps = zp.tile([128, 1], f32, tag="z", name=f"z{i}")
                        for hh in range(2):
                            nc.tensor.matmul(out=z_ps[:],
                                             lhsT=feats[hh][:, i * 128:(i + 1) * 128],
                                             rhs=wf_sb[:, hh:hh + 1],
                                             start=(hh == 0), stop=(hh == 1))
                        nc.vector.tensor_tensor(out=mains_sb[:, 2 * i:2 * i + 1],
                                                in0=z_ps[:],
                                                in1=dpre_sb[:, i:i + 1],
                                                op=mybir.AluOpType.add)

                # ---- P4: local segment-sum of [z, 1] over this core's 128 days ----
                with (
                    tc.tile_pool(name="segsb", bufs=2) as ssp,
                    tc.tile_pool(name="segps", bufs=1, space="PSUM") as pp,
                ):
                    seg_ps = pp.tile([128, 2], f32, name="seg")
                    for i in range(NCHUNK):
                        st_sb = ssp.tile([128, 1], f32, tag="st")
                        nc.sync.dma_start(out=st_sb[:], in_=d_stf[i * 128:(i + 1) * 128, :])
                        oh_sb = ssp.tile([128, SDAYS], f32, tag="oh")
                        nc.vector.tensor_tensor(out=oh_sb[:],
                                                in0=st_sb[:, 0:1].to_broadcast([128, SDAYS]),
                                                in1=iota_sb[:],
                                                op=mybir.AluOpType.is_equal)
                        nc.tensor.matmul(out=seg_ps[:],
                                         lhsT=oh_sb[:],
                                         rhs=mains_sb[:, 2 * i:2 * i + 2],
                                         start=(i == 0), stop=(i == NCHUNK - 1))

                    # ---- P5: finalize mean + linear + sigmoid on 128 local days ----
                    fs = ssp.tile([128, 2], f32, tag="fs")
                    nc.vector.tensor_copy(out=fs[:], in_=seg_ps[:])
                    cnt = ssp.tile([128, 1], f32, tag="cnt")
                    nc.vector.tensor_scalar_max(out=cnt[:], in0=fs[:, 1:2], scalar1=1.0)
                    rcp = ssp.tile([128, 1], f32, tag="rcp")
                    nc.vector.reciprocal(out=rcp[:], in_=cnt[:])
                    dot = ssp.tile([128, 1], f32, tag="dot")
                    nc.vector.tensor_tensor(out=dot[:], in0=fs[:, 0:1], in1=rcp[:],
                                            op=mybir.AluOpType.mult)
                    outsb = ssp.tile([128, 1], f32, tag="osb")
                    nc.scalar.activation(out=outsb[:], in_=dot[:],
                                         func=mybir.ActivationFunctionType.Sigmoid,
                                         bias=bsig_sb[:, 0:1], scale=1.0)
                    nc.sync.dma_start(out=d_out[:], in_=outsb[:])

        if loop_k == 1:
            body()
        else:
            with tc.For_i(0, loop_k) as _i:
                body()

    _split_waits(nc)
    mybir.codegen_inst_isa_subclasses(nc)
    return nc


_NC_CACHE = {}


def _get_nc(loop_k=1):
    if loop_k not in _NC_CACHE:
        _NC_CACHE[loop_k] = _build_nc(loop_k)
    return _NC_CACHE[loop_k]


def _prep_inputs(text, start_times, emb, conv_w, conv_b, W, b):
    bf16 = ml_dtypes.bfloat16
    text = np.asarray(text)[0]              # [N, L]
    st = np.asarray(start_times)[0].astype(np.int64)   # [N], sorted
    emb = np.asarray(emb, dtype=np.float32)
    conv_w = np.asarray(conv_w, dtype=np.float32)
    conv_b = np.asarray(conv_b, dtype=np.float32)
    W = np.asarray(W, dtype=np.float32)
    b = np.asarray(b, dtype=np.float32)

    embp = np.zeros((V, 128), dtype=bf16)
    embp[:, :E] = emb.astype(bf16)

    w01 = np.zeros((128, H), dtype=bf16)
    w01[:64, :] = conv_w[:, :, 0].T.astype(bf16)
    w01[64:, :] = conv_w[:, :, 1].T.astype(bf16)
    w2 = np.ascontiguousarray(conv_w[:, :, 2].T.astype(bf16))
    convb2 = np.ascontiguousarray(conv_b.reshape(2, 128).T.astype(np.float32))
    wf = np.ascontiguousarray(W[1:H + 1, 0].reshape(2, 128).T.astype(np.float32))
    bsig = np.full((128, 1), b[0], np.float32)

    delta_g = np.concatenate([[0.0], np.diff(st).astype(np.float32)]).astype(np.float32)
    dpre_g = delta_g * W[0, 0]

    tok = text.astype(np.int16)             # V=30000 < 2**15

    # day-range sharding: core c owns days [128c, 128c+128); st is sorted so
    # its notes are the contiguous range [lo, hi)
    bounds = np.searchsorted(st, np.arange(0, S + 1, SDAYS))
    in_maps = []
    for c in range(NCORES):
        lo, hi = int(bounds[c]), int(bounds[c + 1])
        n = hi - lo
        assert n <= CAP, f"core {c}: {n} notes > CAP {CAP}"
        ctok = np.zeros((CAP, L), np.int16)
        ctok[:n] = tok[lo:hi]
        cst = np.full((CAP, 1), -1.0, np.float32)
        cst[:n, 0] = st[lo:hi].astype(np.float32)
        cdpre = np.zeros((CAP,), np.float32)
        cdpre[:n] = dpre_g[lo:hi]
        iota = np.tile(np.arange(c * SDAYS, (c + 1) * SDAYS, dtype=np.float32),
                       (128, 1))

        t = ctok.reshape(NBLK, BLK_TOK // GCHUNK, GCHUNK)
        # per-chunk wrap: [32, 16] -> [16, 32], tiled to 128 partitions
        w = t.reshape(NBLK, BLK_TOK // GCHUNK, GCHUNK // 16, 16)
        w = w.transpose(0, 1, 3, 2)                 # [NBLK, NGC, 16, GCHUNK//16]
        w = np.tile(w, (1, 1, 8, 1))                # [NBLK, NGC, 128, GCHUNK//16]
        idx = np.ascontiguousarray(
            w.transpose(0, 2, 1, 3).reshape(NBLK, 128, BLK_TOK // 16))
        in_maps.append({
            "embp": embp,
            "idx": idx,
            "stf": np.ascontiguousarray(cst),
            "dpre": np.ascontiguousarray(cdpre.reshape(NCHUNK, 128).T),
            "w01": w01,
            "w2": w2,
            "convb2": convb2,
            "wf": wf,
            "iota": iota,
            "bsig": bsig,
        })
    return in_maps


def kernel(**inputs) -> np.ndarray:
    nc = _get_nc()
    in_maps = _prep_inputs(**inputs)
    res = run_bass_kernel_spmd(nc, in_maps, list(range(NCORES))).results
    out = np.concatenate([res[c]["out"] for c in range(NCORES)], axis=0)
    return out.astype(np.float32)


if __name__ == "__main__":
    import jax
    import reference
    cpu = jax.devices("cpu")[0]
    with jax.default_device(cpu):
        ins = {k: np.asarray(v) for k, v in reference.setup_inputs().items()}
        exp = np.asarray(reference.reference(**reference.setup_inputs()))
    got = kernel(**ins)
    err = np.abs(got - exp).max()
    rel = err / max(np.abs(exp).max(), 1e-9)
    print("max abs err:", err, "rel:", rel)
